# revision 12
# baseline (speedup 1.0000x reference)
"""Trainium2 Bass kernel for nn_Block_rel (dense transformer block with rel_pos_bias).

Sharding: 8 cores = 4 batches x 2 query-row halves. Each core computes the full
block for its 256 query rows of one batch element. No collectives: k/v
projections are recomputed per core (cheap), attention rows are independent.

Host prep per core:
  - x rows permuted own-half-first (so the kernel always works on rows 0..255);
    rel's key axis (j) is permuted identically.
  - rel passed twice in bf16: natural layout [jt, jp, i, c] (for attn@rel, j on
    partitions) and transposed [i, c, j] (for q@rel, c on partitions).
  - all weights pre-transposed to contraction-on-partitions layouts, bf16.
  - 24 permutation matrices that scatter pair-packed bias rows into the
    head-pair-packed score layout via matmul accumulation.
"""
import numpy as np
from contextlib import ExitStack

import concourse.bass as bass
import concourse.bacc as bacc
import concourse.tile as tile
from concourse import mybir
from concourse.bass_utils import run_bass_kernel_spmd
from concourse.masks import make_identity

BF16 = mybir.dt.bfloat16
F32 = mybir.dt.float32

B, N, D, H = 4, 512, 384, 6
HD = D // H          # 64
FF = 4 * D           # 1536
I = N // 2           # 256 own query rows per core
P = 128
EPS = 1e-5
NCORES = 8

_NP_BF16 = mybir.dt.np(BF16)


def _build_perm() -> np.ndarray:
    """perm[hp*8+gpos][k, m] scatters bias rows (pair-packed, 4 pairs/bank) into
    score rows (64*hs + i-within-64-block) for head-pair hp."""
    perm = np.zeros((24, P, P), np.float32)
    for hp in range(3):
        for gpos in range(8):
            for pp in range(4):
                for ip in range(2):
                    for hs in range(2):
                        h = 2 * hp + hs
                        k = 32 * pp + 6 * ip + h
                        m = 64 * hs + 8 * gpos + 2 * pp + ip
                        perm[hp * 8 + gpos, k, m] = 1.0
    return perm


def build_nc():
    nc = bacc.Bacc("TRN2", target_bir_lowering=False, debug=False)

    # ---- DRAM params (per-core shard shapes) ----
    xp = nc.dram_tensor("xp", [N, D], F32, kind="ExternalInput")
    relT = nc.dram_tensor("relT", [I, HD, N], BF16, kind="ExternalInput")
    relN = nc.dram_tensor("relN", [4, P, I, HD], BF16, kind="ExternalInput")
    wqt = nc.dram_tensor("wqt", [D, D], BF16, kind="ExternalInput")
    wkt = nc.dram_tensor("wkt", [D, D], BF16, kind="ExternalInput")
    wvt = nc.dram_tensor("wvt", [D, D], BF16, kind="ExternalInput")
    wot = nc.dram_tensor("wot", [D, D], BF16, kind="ExternalInput")
    w1t = nc.dram_tensor("w1t", [D, FF], BF16, kind="ExternalInput")
    w2t = nc.dram_tensor("w2t", [FF, D], BF16, kind="ExternalInput")
    perm = nc.dram_tensor("perm", [24, P, P], BF16, kind="ExternalInput")
    ln1w = nc.dram_tensor("ln1w", [D], F32, kind="ExternalInput")
    ln1b = nc.dram_tensor("ln1b", [D], F32, kind="ExternalInput")
    ln2w = nc.dram_tensor("ln2w", [D], F32, kind="ExternalInput")
    ln2b = nc.dram_tensor("ln2b", [D], F32, kind="ExternalInput")
    bo = nc.dram_tensor("bo", [D], F32, kind="ExternalInput")
    b1 = nc.dram_tensor("b1", [FF], F32, kind="ExternalInput")
    b2 = nc.dram_tensor("b2", [D], F32, kind="ExternalInput")
    out = nc.dram_tensor("out", [I, D], F32, kind="ExternalOutput")

    def bcast(t, dim):
        return bass.AP(tensor=t, offset=0, ap=[[0, P], [1, dim]])

    with tile.TileContext(nc) as tc, ExitStack() as ctx:
        singles = ctx.enter_context(tc.tile_pool(name="singles", bufs=1))
        relt_pool = ctx.enter_context(tc.tile_pool(name="relt", bufs=3))
        reln_pool = ctx.enter_context(tc.tile_pool(name="reln", bufs=3))
        bias_pool = ctx.enter_context(tc.tile_pool(name="biassb", bufs=3))
        small = ctx.enter_context(tc.tile_pool(name="small", bufs=4))
        arel_pool = ctx.enter_context(tc.tile_pool(name="arelsb", bufs=3))
        # PSUM budget (8 banks): sc x3 + rstream x2 + pst x2 + pswork x1
        ps_sc = ctx.enter_context(tc.tile_pool(name="ps_sc", bufs=3, space="PSUM"))
        ps_rs = ctx.enter_context(tc.tile_pool(name="ps_rs", bufs=2, space="PSUM"))
        ps_t = ctx.enter_context(tc.tile_pool(name="ps_t", bufs=2, space="PSUM"))
        ps_w = ctx.enter_context(tc.tile_pool(name="ps_w", bufs=1, space="PSUM"))

        # ---- persistent SBUF tensors ----
        x_sb = singles.tile([P, 4, D], F32)
        wqt_sb = singles.tile([P, 3, D], BF16)
        wkt_sb = singles.tile([P, 3, D], BF16)
        wvt_sb = singles.tile([P, 3, D], BF16)
        wot_sb = singles.tile([P, 3, D], BF16)
        wot_sb2 = singles.tile([HD, 6, D], BF16)
        w1t_sb = singles.tile([P, 3, FF], BF16)
        w2t_sb = singles.tile([P, 12, D], BF16)
        perm_sb = singles.tile([P, 24, P], BF16)
        ln1w_sb = singles.tile([P, D], F32)
        ln1b_sb = singles.tile([P, D], F32)
        ln2w_sb = singles.tile([P, D], F32)
        ln2b_sb = singles.tile([P, D], F32)
        bo_sb = singles.tile([P, D], F32)
        b2_sb = singles.tile([P, D], F32)
        b1_sb = singles.tile([P, 12], F32)
        eps_sb = singles.tile([P, 1], F32)
        ident = singles.tile([P, P], BF16)

        xn_sb = singles.tile([P, 4, D], BF16)
        xnT = singles.tile([P, 3, N], BF16)
        kT = singles.tile([P, 3, N], BF16)
        v_sb = singles.tile([P, 4, D], BF16)
        qT = singles.tile([P, 3, I], BF16)
        lhsT_sc = singles.tile([P, 3, 4, P], BF16)
        lhsT_qr = singles.tile([P, P, 32], BF16)
        attn_sb = singles.tile([P, 3, 4, N], BF16)   # (hs,i') x (hp, ib, j)
        attnT = singles.tile([P, 4, 6 * I], BF16)    # j x (jt, 6i+h)
        aoT_alt = singles.tile([HD, 6, I], BF16)     # c x (h, i)  [attn@rel out]
        avT = singles.tile([P, 3, I], BF16)          # e x i       [attn@v out]
        x2_sb = singles.tile([P, 2, D], F32)
        x2n_sb = singles.tile([P, 2, D], BF16)
        x2nT = singles.tile([P, 3, I], BF16)
        h1g = singles.tile([P, 12, I], BF16)
        out_sb = singles.tile([P, 2, D], F32)
        rz_sb = singles.tile([P, 3, 4], F32)         # 1/Z per (hp, ib)

        # ---- loads ----
        nc.sync.dma_start(out=x_sb[:], in_=xp.ap().rearrange("(t p) d -> p t d", p=P))
        nc.sync.dma_start(out=wqt_sb[:], in_=wqt.ap().rearrange("(t p) e -> p t e", p=P))
        nc.sync.dma_start(out=wkt_sb[:], in_=wkt.ap().rearrange("(t p) e -> p t e", p=P))
        nc.sync.dma_start(out=wvt_sb[:], in_=wvt.ap().rearrange("(t p) e -> p t e", p=P))
        nc.sync.dma_start(out=wot_sb[:], in_=wot.ap().rearrange("(t p) e -> p t e", p=P))
        nc.sync.dma_start(out=wot_sb2[:], in_=wot.ap().rearrange("(h c) d -> c h d", c=HD))
        nc.sync.dma_start(out=w1t_sb[:], in_=w1t.ap().rearrange("(t p) e -> p t e", p=P))
        nc.sync.dma_start(out=w2t_sb[:], in_=w2t.ap().rearrange("(t p) e -> p t e", p=P))
        nc.sync.dma_start(out=perm_sb[:], in_=perm.ap().rearrange("n k m -> k n m"))
        nc.sync.dma_start(out=ln1w_sb[:], in_=bcast(ln1w, D))
        nc.sync.dma_start(out=ln1b_sb[:], in_=bcast(ln1b, D))
        nc.sync.dma_start(out=ln2w_sb[:], in_=bcast(ln2w, D))
        nc.sync.dma_start(out=ln2b_sb[:], in_=bcast(ln2b, D))
        nc.sync.dma_start(out=bo_sb[:], in_=bcast(bo, D))
        nc.sync.dma_start(out=b2_sb[:], in_=bcast(b2, D))
        nc.sync.dma_start(out=b1_sb[:], in_=b1.ap().rearrange("(t p) -> p t", p=P))
        nc.vector.memset(eps_sb[:], EPS)
        make_identity(nc, ident[:])
        nc.gpsimd.memset(lhsT_sc[:], 0.0)
        nc.gpsimd.memset(lhsT_qr[:], 0.0)

        # ---- LayerNorm 1 -> xn (bf16) ----
        def layer_norm(src_f32, w_b, b_b, dst_bf16, ntiles):
            for t in range(ntiles):
                stats = small.tile([P, 6], F32, tag="lnstats")
                mv = small.tile([P, 2], F32, tag="lnmv")
                nc.vector.bn_stats(out=stats[:], in_=src_f32[:, t, :])
                nc.vector.bn_aggr(out=mv[:], in_=stats[:])
                rstd = small.tile([P, 1], F32, tag="lnrstd")
                nc.scalar.activation(out=rstd[:], in_=mv[:, 1:2],
                                     func=mybir.ActivationFunctionType.Sqrt,
                                     bias=eps_sb[:], scale=1.0)
                nc.vector.reciprocal(out=rstd[:], in_=rstd[:])
                tmp = small.tile([P, D], F32, tag="lntmp")
                nc.vector.tensor_scalar(out=tmp[:], in0=src_f32[:, t, :],
                                        scalar1=mv[:, 0:1], scalar2=rstd[:],
                                        op0=mybir.AluOpType.subtract,
                                        op1=mybir.AluOpType.mult)
                nc.vector.tensor_tensor(out=tmp[:], in0=tmp[:], in1=w_b[:],
                                        op=mybir.AluOpType.mult)
                nc.vector.tensor_tensor(out=dst_bf16[:, t, :], in0=tmp[:], in1=b_b[:],
                                        op=mybir.AluOpType.add)

        layer_norm(x_sb, ln1w_sb, ln1b_sb, xn_sb, 4)

        # ---- xnT via PE transpose ----
        for dt in range(3):
            for nt in range(4):
                pt = ps_t.tile([P, P], BF16, tag="pst")
                nc.tensor.transpose(pt[:], xn_sb[:, nt, dt * P:(dt + 1) * P], ident[:])
                nc.scalar.copy(out=xnT[:, dt, nt * P:(nt + 1) * P], in_=pt[:])

        # ---- projections ----
        for et in range(3):
            ps = ps_w.tile([P, N], F32, tag="pswork")
            for dt in range(3):
                nc.tensor.matmul(ps[:], wkt_sb[:, dt, et * P:(et + 1) * P],
                                 xnT[:, dt, :], start=(dt == 0), stop=(dt == 2))
            nc.scalar.copy(out=kT[:, et, :], in_=ps[:])
        for nt in range(4):
            ps = ps_w.tile([P, N], F32, tag="pswork")
            for dt in range(3):
                nc.tensor.matmul(ps[:, 0:D], xnT[:, dt, nt * P:(nt + 1) * P],
                                 wvt_sb[:, dt, :], start=(dt == 0), stop=(dt == 2))
            nc.scalar.copy(out=v_sb[:, nt, :], in_=ps[:, 0:D])
        for et in range(3):
            ps = ps_w.tile([P, N], F32, tag="pswork")
            for dt in range(3):
                nc.tensor.matmul(ps[:, 0:I], wqt_sb[:, dt, et * P:(et + 1) * P],
                                 xnT[:, dt, 0:I], start=(dt == 0), stop=(dt == 2))
            nc.scalar.mul(out=qT[:, et, :], in_=ps[:, 0:I], mul=float(HD) ** -0.5)

        # ---- block-diag lhsT builds ----
        # scores: lhsT_sc[64hs+c, hp, ib, 64hs+i'] = qT[128hp+64hs+c, 64ib+i']
        for hp in range(3):
            for hs in range(2):
                src = qT[64 * hs:64 * hs + 64, hp, :].rearrange("p (b i) -> p b i", b=4)
                dst = lhsT_sc[64 * hs:64 * hs + 64, hp, :, 64 * hs:64 * hs + 64]
                nc.vector.tensor_copy(out=dst, in_=src)
        # q@rel: lhsT_qr[64ip+c, p, 6ip+h] = qT[64h+c, 2p+ip] * (pair packing)
        qT_pair = qT[:].rearrange("p t (i two) -> p t i two", two=2)
        for h in range(H):
            for ip in range(2):
                src = qT_pair[64 * (h % 2):64 * (h % 2) + 64, h // 2, :, ip]
                dst = lhsT_qr[64 * ip:64 * ip + 64, :, 6 * ip + h]
                nc.vector.tensor_copy(out=dst, in_=src)

        # ---- attention streaming over i-blocks ----
        for ib in range(4):
            # scores psum tiles for this ib (3 head pairs), q@k first
            sc_ps = []
            for hp in range(3):
                sct = ps_sc.tile([P, N], F32, tag="sc")
                sc_ps.append(sct)
            for hp in range(3):
                nc.tensor.matmul(sc_ps[hp][:], lhsT_sc[:, hp, ib, :], kT[:, hp, :],
                                 start=True, stop=False, skip_group_check=True)
            for gg in range(8):
                g = 8 * ib + gg
                # load relT for the 4 pairs of this group
                rt = relt_pool.tile([P, 4, N], BF16)
                for ip in range(2):
                    src = bass.AP(tensor=relT, offset=(8 * g + ip) * HD * N,
                                  ap=[[N, HD], [2 * HD * N, 4], [1, N]])
                    nc.sync.dma_start(out=rt[64 * ip:64 * ip + 64, :, :], in_=src)
                bias_ps = ps_rs.tile([P, N], F32, tag="rstream")
                for pp in range(4):
                    p = 4 * g + pp
                    nc.tensor.matmul(bias_ps[32 * pp:32 * pp + 32, :],
                                     lhsT_qr[:, p, :], rt[:, pp, :],
                                     start=True, stop=True, skip_group_check=True,
                                     tile_position=(0, 32 * pp))
                bias_sb = bias_pool.tile([P, N], BF16)
                nc.scalar.copy(out=bias_sb[:], in_=bias_ps[:])
                for hp in range(3):
                    nc.tensor.matmul(sc_ps[hp][:], perm_sb[:, 8 * hp + gg, :],
                                     bias_sb[:], start=False, stop=(gg == 7),
                                     skip_group_check=True)
            # softmax (no max subtraction: logits are bounded for this problem)
            for hp in range(3):
                zcol = small.tile([P, 1], F32, tag="zcol")
                nc.scalar.activation(out=attn_sb[:, hp, ib, :], in_=sc_ps[hp][:],
                                     func=mybir.ActivationFunctionType.Exp,
                                     accum_out=zcol[:])
                nc.vector.reciprocal(out=rz_sb[:, hp, ib:ib + 1], in_=zcol[:])
                nc.vector.tensor_scalar_mul(attn_sb[:, hp, ib, :],
                                            attn_sb[:, hp, ib, :],
                                            rz_sb[:, hp, ib:ib + 1])
            # attnT: [j, 6i+h] for i in this ib
            for hp in range(3):
                for jt in range(4):
                    pt = ps_t.tile([P, P], BF16, tag="pst")
                    nc.tensor.transpose(pt[:], attn_sb[:, hp, ib, jt * P:(jt + 1) * P],
                                        ident[:])
                    # pt rows j, cols (hs,i') -> attnT col 384*ib + 6*i' + 2hp + hs
                    at_base = attnT[:, jt, :]
                    dst = bass.AP(tensor=at_base.tensor,
                                  offset=at_base.offset + 384 * ib + 2 * hp,
                                  ap=[at_base.ap[0], [1, 2], [6, 64]])
                    pt_base = pt[:]
                    src = bass.AP(tensor=pt_base.tensor, offset=pt_base.offset,
                                  ap=[pt_base.ap[0], [64, 2], [1, 64]])
                    nc.scalar.copy(out=dst, in_=src)
            # attn@rel for groups of this ib
            for gg in range(8):
                g = 8 * ib + gg
                rn = reln_pool.tile([P, 4, 8, HD], BF16)
                src = bass.AP(tensor=relN, offset=8 * g * HD,
                              ap=[[I * HD, P], [P * I * HD, 4], [HD, 8], [1, HD]])
                nc.sync.dma_start(out=rn[:], in_=src)
                ar_ps = ps_rs.tile([P, N], F32, tag="rstream")
                for jt in range(4):
                    nc.tensor.matmul(ar_ps[0:48, :],
                                     attnT[:, jt, 48 * g:48 * g + 48],
                                     rn[:, jt, :, :], start=(jt == 0), stop=(jt == 3),
                                     skip_group_check=True)
                ar_sb = arel_pool.tile([48, N], BF16, tag="arsb")
                nc.scalar.copy(out=ar_sb[:], in_=ar_ps[0:48, :])
                for ct in range(4):
                    pt = ps_t.tile([P, P], BF16, tag="pst")
                    nc.tensor.transpose(pt[:, 0:48], ar_sb[:, ct * P:(ct + 1) * P],
                                        ident[0:48, 0:48])
                    art = arel_pool.tile([P, 48], BF16, tag="artsb")
                    nc.scalar.copy(out=art[:], in_=pt[:, 0:48])
                    for nd in range(2):
                        n = 2 * ct + nd
                        i = 8 * g + n
                        blk = art[64 * nd:64 * nd + 64, 6 * n:6 * n + 6]
                        if n % 2 == 0:
                            nc.vector.tensor_copy(out=aoT_alt[:, :, i], in_=blk)
                        else:
                            nc.scalar.copy(out=aoT_alt[:, :, i], in_=blk)

        # ---- attn@v -> avT ----
        for it in range(2):
            ps = ps_w.tile([P, N], F32, tag="pswork")
            for h in range(H):
                for jt in range(4):
                    at_base = attnT[:, jt, :]
                    lhs = bass.AP(tensor=at_base.tensor,
                                  offset=at_base.offset + 768 * it + h,
                                  ap=[at_base.ap[0], [6, P]])
                    nc.tensor.matmul(ps[:, 64 * h:64 * h + 64], lhs,
                                     v_sb[:, jt, 64 * h:64 * h + 64],
                                     start=(jt == 0), stop=(jt == 3),
                                     skip_group_check=True)
            av = small.tile([P, D], BF16, tag="avsb")
            nc.scalar.copy(out=av[:], in_=ps[:, 0:D])
            for dt in range(3):
                pt = ps_t.tile([P, P], BF16, tag="pst")
                nc.tensor.transpose(pt[:], av[:, dt * P:(dt + 1) * P], ident[:])
                nc.scalar.copy(out=avT[:, dt, it * P:(it + 1) * P], in_=pt[:])

        # ---- Wo projection + residual ----
        for it in range(2):
            ps = ps_w.tile([P, N], F32, tag="pswork")
            for et in range(3):
                nc.tensor.matmul(ps[:, 0:D], avT[:, et, it * P:(it + 1) * P],
                                 wot_sb[:, et, :], start=(et == 0), stop=False,
                                 skip_group_check=True)
            for h in range(H):
                nc.tensor.matmul(ps[:, 0:D], aoT_alt[:, h, it * P:(it + 1) * P],
                                 wot_sb2[:, h, :],
                                 start=False, stop=(h == H - 1),
                                 skip_group_check=True)
            tmp = small.tile([P, D], F32, tag="res")
            nc.vector.tensor_tensor(out=tmp[:], in0=ps[:, 0:D], in1=bo_sb[:],
                                    op=mybir.AluOpType.add)
            nc.vector.tensor_tensor(out=x2_sb[:, it, :], in0=tmp[:],
                                    in1=x_sb[:, it, :], op=mybir.AluOpType.add)

        # ---- LN2 + MLP ----
        layer_norm(x2_sb, ln2w_sb, ln2b_sb, x2n_sb, 2)
        for dt in range(3):
            for it in range(2):
                pt = ps_t.tile([P, P], BF16, tag="pst")
                nc.tensor.transpose(pt[:], x2n_sb[:, it, dt * P:(dt + 1) * P], ident[:])
                nc.scalar.copy(out=x2nT[:, dt, it * P:(it + 1) * P], in_=pt[:])
        for ft in range(12):
            ps = ps_w.tile([P, N], F32, tag="pswork")
            for dt in range(3):
                nc.tensor.matmul(ps[:, 0:I], w1t_sb[:, dt, ft * P:(ft + 1) * P],
                                 x2nT[:, dt, :], start=(dt == 0), stop=(dt == 2))
            nc.scalar.activation(out=h1g[:, ft, :], in_=ps[:, 0:I],
                                 func=mybir.ActivationFunctionType.Gelu,
                                 bias=b1_sb[:, ft:ft + 1], scale=1.0)
        for it in range(2):
            ps = ps_w.tile([P, N], F32, tag="pswork")
            for ft in range(12):
                nc.tensor.matmul(ps[:, 0:D], h1g[:, ft, it * P:(it + 1) * P],
                                 w2t_sb[:, ft, :], start=(ft == 0), stop=(ft == 11))
            tmp = small.tile([P, D], F32, tag="res")
            nc.vector.tensor_tensor(out=tmp[:], in0=ps[:, 0:D], in1=b2_sb[:],
                                    op=mybir.AluOpType.add)
            nc.vector.tensor_tensor(out=out_sb[:, it, :], in0=tmp[:],
                                    in1=x2_sb[:, it, :], op=mybir.AluOpType.add)

        nc.sync.dma_start(out=out.ap().rearrange("(t p) d -> p t d", p=P),
                          in_=out_sb[:])

    nc.compile()
    return nc


_NC_CACHE = None


def _get_nc():
    global _NC_CACHE
    if _NC_CACHE is None:
        _NC_CACHE = build_nc()
    return _NC_CACHE


def kernel(x, rel_pos_bias, ln1_w, ln1_b, ln2_w, ln2_b, Wq, Wk, Wv, Wo, bo,
           W1, b1, W2, b2):
    nc = _get_nc()
    perm_f = _build_perm()
    common = {
        "wqt": np.ascontiguousarray(Wq.T).astype(_NP_BF16),
        "wkt": np.ascontiguousarray(Wk.T).astype(_NP_BF16),
        "wvt": np.ascontiguousarray(Wv.T).astype(_NP_BF16),
        "wot": np.ascontiguousarray(Wo.T).astype(_NP_BF16),
        "w1t": np.ascontiguousarray(W1.T).astype(_NP_BF16),
        "w2t": np.ascontiguousarray(W2.T).astype(_NP_BF16),
        "perm": perm_f.astype(_NP_BF16),
        "ln1w": np.asarray(ln1_w, np.float32), "ln1b": np.asarray(ln1_b, np.float32),
        "ln2w": np.asarray(ln2_w, np.float32), "ln2b": np.asarray(ln2_b, np.float32),
        "bo": np.asarray(bo, np.float32), "b1": np.asarray(b1, np.float32),
        "b2": np.asarray(b2, np.float32),
    }
    in_maps = []
    for core in range(NCORES):
        b, ih = core // 2, core % 2
        own = slice(ih * I, (ih + 1) * I)
        othr = slice((1 - ih) * I, (2 - ih) * I)
        permrows = np.r_[ih * I:(ih + 1) * I, (1 - ih) * I:(2 - ih) * I]
        xp = np.ascontiguousarray(np.asarray(x[b], np.float32)[permrows])
        rel = np.asarray(rel_pos_bias[b], np.float32)[own][:, permrows, :]
        rel_bf = rel.astype(_NP_BF16)
        relT = np.ascontiguousarray(rel_bf.transpose(0, 2, 1))
        relN = np.ascontiguousarray(
            rel_bf.transpose(1, 0, 2).reshape(4, P, I, HD))
        in_maps.append({**common, "xp": xp, "relT": relT, "relN": relN})
    res = run_bass_kernel_spmd(nc, in_maps, core_ids=list(range(NCORES)))
    out = np.empty((B, N, D), np.float32)
    for core in range(NCORES):
        b, ih = core // 2, core % 2
        out[b, ih * I:(ih + 1) * I] = res.results[core]["out"]
    return out


# revision 21
# speedup vs baseline: 33360.3935x; 33360.3935x over previous
"""Trainium2 Bass kernel for nn_Block_rel (dense transformer block with rel_pos_bias).

Sharding: 8 cores = 4 batches x 2 query-row halves. Each core computes the full
block for its 256 query rows of one batch element. No collectives: k/v
projections are recomputed per core (cheap), attention rows are independent.

Host prep per core:
  - x rows permuted own-half-first (so the kernel always works on rows 0..255);
    rel's key axis (j) is permuted identically.
  - rel passed twice in bf16: natural layout [jt, jp, i, c] (for attn@rel, j on
    partitions) and transposed [i, c, j] (for q@rel, c on partitions).
  - all weights pre-transposed to contraction-on-partitions layouts, bf16.
  - 24 permutation matrices that scatter pair-packed bias rows into the
    head-pair-packed score layout via matmul accumulation.
"""
import numpy as np
from contextlib import ExitStack

import concourse.bass as bass
import concourse.bacc as bacc
import concourse.tile as tile
from concourse import mybir
from concourse.bass_utils import run_bass_kernel_spmd
from concourse.masks import make_identity

BF16 = mybir.dt.bfloat16
F32 = mybir.dt.float32

B, N, D, H = 4, 512, 384, 6
HD = D // H          # 64
FF = 4 * D           # 1536
I = N // 2           # 256 own query rows per core
P = 128
EPS = 1e-5
NCORES = 8

_NP_BF16 = mybir.dt.np(BF16)


def _build_perm() -> np.ndarray:
    """perm[hp*8+gpos][k, m] scatters bias rows (pair-packed, 4 pairs/bank) into
    score rows (64*hs + i-within-64-block) for head-pair hp."""
    perm = np.zeros((24, P, P), np.float32)
    for hp in range(3):
        for gpos in range(8):
            for pp in range(4):
                for ip in range(2):
                    for hs in range(2):
                        h = 2 * hp + hs
                        k = 32 * pp + 6 * ip + h
                        m = 64 * hs + 8 * gpos + 2 * pp + ip
                        perm[hp * 8 + gpos, k, m] = 1.0
    return perm


def build_nc():
    nc = bacc.Bacc("TRN2", target_bir_lowering=False, debug=False)

    # ---- DRAM params (per-core shard shapes) ----
    xp = nc.dram_tensor("xp", [N, D], F32, kind="ExternalInput")
    relT = nc.dram_tensor("relT", [I, HD, N], BF16, kind="ExternalInput")
    relN = nc.dram_tensor("relN", [4, P, I, HD], BF16, kind="ExternalInput")
    wqt = nc.dram_tensor("wqt", [D, D], BF16, kind="ExternalInput")
    wkt = nc.dram_tensor("wkt", [D, D], BF16, kind="ExternalInput")
    wvt = nc.dram_tensor("wvt", [D, D], BF16, kind="ExternalInput")
    wot = nc.dram_tensor("wot", [D, D], BF16, kind="ExternalInput")
    w1t = nc.dram_tensor("w1t", [D, FF], BF16, kind="ExternalInput")
    w2t = nc.dram_tensor("w2t", [FF, D], BF16, kind="ExternalInput")
    perm = nc.dram_tensor("perm", [24, P, P], BF16, kind="ExternalInput")
    ln1w = nc.dram_tensor("ln1w", [D], F32, kind="ExternalInput")
    ln1b = nc.dram_tensor("ln1b", [D], F32, kind="ExternalInput")
    ln2w = nc.dram_tensor("ln2w", [D], F32, kind="ExternalInput")
    ln2b = nc.dram_tensor("ln2b", [D], F32, kind="ExternalInput")
    bo = nc.dram_tensor("bo", [D], F32, kind="ExternalInput")
    b1 = nc.dram_tensor("b1", [FF], F32, kind="ExternalInput")
    b2 = nc.dram_tensor("b2", [D], F32, kind="ExternalInput")
    out = nc.dram_tensor("out", [I, D], F32, kind="ExternalOutput")

    def bcast(t, dim):
        return bass.AP(tensor=t, offset=0, ap=[[0, P], [1, dim]])

    with tile.TileContext(nc) as tc, ExitStack() as ctx:
        singles = ctx.enter_context(tc.tile_pool(name="singles", bufs=1))
        relt_pool = ctx.enter_context(tc.tile_pool(name="relt", bufs=7))
        reln_pool = ctx.enter_context(tc.tile_pool(name="reln", bufs=9))
        bias_pool = ctx.enter_context(tc.tile_pool(name="biassb", bufs=6))
        small = ctx.enter_context(tc.tile_pool(name="small", bufs=3))
        arel_pool = ctx.enter_context(tc.tile_pool(name="arelsb", bufs=3))
        # PSUM budget (8 banks): sc x3 + rstream x2 + pst x2 + pswork x1
        ps_sc = ctx.enter_context(tc.tile_pool(name="ps_sc", bufs=3, space="PSUM"))
        ps_rs = ctx.enter_context(tc.tile_pool(name="ps_rs", bufs=2, space="PSUM"))
        ps_t = ctx.enter_context(tc.tile_pool(name="ps_t", bufs=2, space="PSUM"))
        ps_w = ctx.enter_context(tc.tile_pool(name="ps_w", bufs=1, space="PSUM"))

        # ---- persistent SBUF tensors ----
        x_sb = singles.tile([P, 4, D], F32)
        wqt_sb = singles.tile([P, 3, D], BF16)
        wkt_sb = singles.tile([P, 3, D], BF16)
        wvt_sb = singles.tile([P, 3, D], BF16)
        wot_sb = singles.tile([P, 3, D], BF16)
        wot_sb2 = singles.tile([HD, 6, D], BF16)
        w1t_sb = singles.tile([P, 3, FF], BF16)
        w2t_sb = singles.tile([P, 12, D], BF16)
        perm_sb = singles.tile([P, 24, P], BF16)
        ln1w_sb = singles.tile([P, D], F32)
        ln1b_sb = singles.tile([P, D], F32)
        ln2w_sb = singles.tile([P, D], F32)
        ln2b_sb = singles.tile([P, D], F32)
        bo_sb = singles.tile([P, D], F32)
        b2_sb = singles.tile([P, D], F32)
        b1_sb = singles.tile([P, 12], F32)
        eps_sb = singles.tile([P, 1], F32)
        ident = singles.tile([P, P], BF16)

        xn_sb = singles.tile([P, 4, D], BF16)
        xnT = singles.tile([P, 3, N], BF16)
        kT = singles.tile([P, 3, N], BF16)
        v_sb = singles.tile([P, 4, D], BF16)
        qT = singles.tile([P, 3, I], BF16)
        lhsT_sc = singles.tile([P, 3, 4, P], BF16)
        lhsT_qr = singles.tile([P, P, 32], BF16)
        attn_sb = singles.tile([P, 3, 4, N], BF16)   # (hs,i') x (hp, ib, j)
        attnT = singles.tile([P, 4, 6 * I], BF16)    # j x (jt, 6i+h)
        aoT_alt = singles.tile([HD, 6, I], BF16)     # c x (h, i)  [attn@rel out]
        avT = singles.tile([P, 3, I], BF16)          # e x i       [attn@v out]
        x2_sb = singles.tile([P, 2, D], F32)
        x2n_sb = singles.tile([P, 2, D], BF16)
        x2nT = singles.tile([P, 3, I], BF16)
        h1g = singles.tile([P, 12, I], BF16)
        out_sb = singles.tile([P, 2, D], F32)
        rz_sb = singles.tile([P, 3, 4], F32)         # 1/Z per (hp, ib)

        # ---- loads ----
        nc.sync.dma_start(out=x_sb[:], in_=xp.ap().rearrange("(t p) d -> p t d", p=P))
        nc.sync.dma_start(out=ln1w_sb[:], in_=bcast(ln1w, D))
        nc.sync.dma_start(out=ln1b_sb[:], in_=bcast(ln1b, D))
        nc.sync.dma_start(out=wqt_sb[:], in_=wqt.ap().rearrange("(t p) e -> p t e", p=P))
        nc.sync.dma_start(out=wkt_sb[:], in_=wkt.ap().rearrange("(t p) e -> p t e", p=P))
        nc.sync.dma_start(out=wvt_sb[:], in_=wvt.ap().rearrange("(t p) e -> p t e", p=P))
        nc.sync.dma_start(out=perm_sb[:], in_=perm.ap().rearrange("n k m -> k n m"))
        nc.sync.dma_start(out=ln2w_sb[:], in_=bcast(ln2w, D))
        nc.sync.dma_start(out=ln2b_sb[:], in_=bcast(ln2b, D))
        nc.sync.dma_start(out=bo_sb[:], in_=bcast(bo, D))
        nc.sync.dma_start(out=b2_sb[:], in_=bcast(b2, D))
        nc.sync.dma_start(out=b1_sb[:], in_=b1.ap().rearrange("(t p) -> p t", p=P))
        nc.vector.memset(eps_sb[:], EPS)
        make_identity(nc, ident[:])
        nc.gpsimd.memset(lhsT_sc[:], 0.0)
        nc.gpsimd.memset(lhsT_qr[:], 0.0)

        # ---- LayerNorm 1 -> xn (bf16) ----
        def layer_norm(src_f32, w_b, b_b, dst_bf16, ntiles):
            for t in range(ntiles):
                stats = small.tile([P, 6], F32, tag="lnstats")
                mv = small.tile([P, 2], F32, tag="lnmv")
                nc.vector.bn_stats(out=stats[:], in_=src_f32[:, t, :])
                nc.vector.bn_aggr(out=mv[:], in_=stats[:])
                rstd = small.tile([P, 1], F32, tag="lnrstd")
                nc.scalar.activation(out=rstd[:], in_=mv[:, 1:2],
                                     func=mybir.ActivationFunctionType.Sqrt,
                                     bias=eps_sb[:], scale=1.0)
                nc.vector.reciprocal(out=rstd[:], in_=rstd[:])
                tmp = small.tile([P, D], F32, tag="lntmp")
                nc.vector.tensor_scalar(out=tmp[:], in0=src_f32[:, t, :],
                                        scalar1=mv[:, 0:1], scalar2=rstd[:],
                                        op0=mybir.AluOpType.subtract,
                                        op1=mybir.AluOpType.mult)
                nc.vector.tensor_tensor(out=tmp[:], in0=tmp[:], in1=w_b[:],
                                        op=mybir.AluOpType.mult)
                nc.vector.tensor_tensor(out=dst_bf16[:, t, :], in0=tmp[:], in1=b_b[:],
                                        op=mybir.AluOpType.add)

        layer_norm(x_sb, ln1w_sb, ln1b_sb, xn_sb, 4)
        nc.sync.dma_start(out=wot_sb[:], in_=wot.ap().rearrange("(t p) e -> p t e", p=P))
        nc.sync.dma_start(out=wot_sb2[:], in_=wot.ap().rearrange("(h c) d -> c h d", c=HD))
        nc.sync.dma_start(out=w1t_sb[:], in_=w1t.ap().rearrange("(t p) e -> p t e", p=P))
        nc.sync.dma_start(out=w2t_sb[:], in_=w2t.ap().rearrange("(t p) e -> p t e", p=P))

        # ---- xnT via PE transpose ----
        for dt in range(3):
            for nt in range(4):
                pt = ps_t.tile([P, P], BF16, tag="pst")
                nc.tensor.transpose(pt[:], xn_sb[:, nt, dt * P:(dt + 1) * P], ident[:])
                nc.vector.tensor_copy(out=xnT[:, dt, nt * P:(nt + 1) * P], in_=pt[:])

        # ---- projections ----
        for et in range(3):
            ps = ps_w.tile([P, N], F32, tag="pswork")
            for dt in range(3):
                nc.tensor.matmul(ps[:], wkt_sb[:, dt, et * P:(et + 1) * P],
                                 xnT[:, dt, :], start=(dt == 0), stop=(dt == 2))
            nc.vector.tensor_copy(out=kT[:, et, :], in_=ps[:])
        for nt in range(4):
            ps = ps_w.tile([P, N], F32, tag="pswork")
            for dt in range(3):
                nc.tensor.matmul(ps[:, 0:D], xnT[:, dt, nt * P:(nt + 1) * P],
                                 wvt_sb[:, dt, :], start=(dt == 0), stop=(dt == 2))
            nc.vector.tensor_copy(out=v_sb[:, nt, :], in_=ps[:, 0:D])
        for et in range(3):
            ps = ps_w.tile([P, N], F32, tag="pswork")
            for dt in range(3):
                nc.tensor.matmul(ps[:, 0:I], wqt_sb[:, dt, et * P:(et + 1) * P],
                                 xnT[:, dt, 0:I], start=(dt == 0), stop=(dt == 2))
            nc.vector.tensor_scalar_mul(qT[:, et, :], ps[:, 0:I], float(HD) ** -0.5)

        # ---- block-diag lhsT builds ----
        # scores: lhsT_sc[64hs+c, hp, ib, 64hs+i'] = qT[128hp+64hs+c, 64ib+i']
        for hp in range(3):
            for hs in range(2):
                src = qT[64 * hs:64 * hs + 64, hp, :].rearrange("p (b i) -> p b i", b=4)
                dst = lhsT_sc[64 * hs:64 * hs + 64, hp, :, 64 * hs:64 * hs + 64]
                nc.vector.tensor_copy(out=dst, in_=src)
        # q@rel: lhsT_qr[64ip+c, p, 6ip+h] = qT[64h+c, 2p+ip] * (pair packing)
        qT_pair = qT[:].rearrange("p t (i two) -> p t i two", two=2)
        for h in range(H):
            for ip in range(2):
                src = qT_pair[64 * (h % 2):64 * (h % 2) + 64, h // 2, :, ip]
                dst = lhsT_qr[64 * ip:64 * ip + 64, :, 6 * ip + h]
                nc.vector.tensor_copy(out=dst, in_=src)

        # attn@v -> avT -> Wo -> residual, for one 128-row half (emitted per ib pair)
        def emit_head_merge(it):
            ps = ps_w.tile([P, N], F32, tag="pswork", name=f"psavm{it}")
            for h in range(H):
                for jt in range(4):
                    at_base = attnT[:, jt, :]
                    lhs = bass.AP(tensor=at_base.tensor,
                                  offset=at_base.offset + 768 * it + h,
                                  ap=[at_base.ap[0], [6, P]])
                    nc.tensor.matmul(ps[:, 64 * h:64 * h + 64], lhs,
                                     v_sb[:, jt, 64 * h:64 * h + 64],
                                     start=(jt == 0), stop=(jt == 3),
                                     skip_group_check=True)
            av = small.tile([P, D], BF16, tag="avsb", name=f"av{it}")
            nc.vector.tensor_copy(out=av[:], in_=ps[:, 0:D])
            for dt in range(3):
                pt = ps_t.tile([P, P], BF16, tag="pst", name=f"ptav{it}{dt}")
                nc.tensor.transpose(pt[:], av[:, dt * P:(dt + 1) * P], ident[:])
                nc.vector.tensor_copy(out=avT[:, dt, it * P:(it + 1) * P], in_=pt[:])
            ps2 = ps_w.tile([P, N], F32, tag="pswork", name=f"pswo{it}")
            for et in range(3):
                nc.tensor.matmul(ps2[:, 0:D], avT[:, et, it * P:(it + 1) * P],
                                 wot_sb[:, et, :], start=(et == 0), stop=False,
                                 skip_group_check=True)
            for h in range(H):
                nc.tensor.matmul(ps2[:, 0:D], aoT_alt[:, h, it * P:(it + 1) * P],
                                 wot_sb2[:, h, :],
                                 start=False, stop=(h == H - 1),
                                 skip_group_check=True)
            tmp = small.tile([P, D], F32, tag="res", name=f"res{it}")
            nc.vector.tensor_tensor(out=tmp[:], in0=ps2[:, 0:D], in1=bo_sb[:],
                                    op=mybir.AluOpType.add)
            nc.vector.tensor_tensor(out=x2_sb[:, it, :], in0=tmp[:],
                                    in1=x_sb[:, it, :], op=mybir.AluOpType.add)

        # ---- attention streaming over i-blocks ----
        for ib in range(4):
            # scores psum tiles for this ib (3 head pairs), q@k first
            sc_ps = []
            for hp in range(3):
                sct = ps_sc.tile([P, N], F32, tag="sc")
                sc_ps.append(sct)
            for hp in range(3):
                nc.tensor.matmul(sc_ps[hp][:], lhsT_sc[:, hp, ib, :], kT[:, hp, :],
                                 start=True, stop=False, skip_group_check=True)
            rn_tiles = []
            for gg in range(8):
                g = 8 * ib + gg
                # load relT for the 4 pairs of this group
                rt = relt_pool.tile([P, 4, N], BF16)
                for ip in range(2):
                    src = bass.AP(tensor=relT, offset=(8 * g + ip) * HD * N,
                                  ap=[[N, HD], [2 * HD * N, 4], [1, N]])
                    nc.sync.dma_start(out=rt[64 * ip:64 * ip + 64, :, :], in_=src)
                rn = reln_pool.tile([P, 4, 8, HD], BF16, tag="rn", name=f"rn{ib}{gg}")
                rnsrc = bass.AP(tensor=relN, offset=8 * g * HD,
                                ap=[[I * HD, P], [P * I * HD, 4], [HD, 8], [1, HD]])
                nc.sync.dma_start(out=rn[:], in_=rnsrc)
                rn_tiles.append(rn)
                bias_ps = ps_rs.tile([P, N], F32, tag="rstream")
                for pp in range(4):
                    p = 4 * g + pp
                    nc.tensor.matmul(bias_ps[32 * pp:32 * pp + 32, :],
                                     lhsT_qr[:, p, :], rt[:, pp, :],
                                     start=True, stop=True, skip_group_check=True,
                                     tile_position=(0, 32 * pp))
                bias_sb = bias_pool.tile([P, N], BF16)
                nc.vector.tensor_copy(out=bias_sb[:], in_=bias_ps[:])
                for hp in range(3):
                    nc.tensor.matmul(sc_ps[hp][:], perm_sb[:, 8 * hp + gg, :],
                                     bias_sb[:], start=False, stop=(gg == 7),
                                     skip_group_check=True)
            # softmax (no max subtraction: logits are bounded for this problem)
            for hp in range(3):
                zcol = small.tile([P, 1], F32, tag="zcol")
                nc.scalar.activation(out=attn_sb[:, hp, ib, :], in_=sc_ps[hp][:],
                                     func=mybir.ActivationFunctionType.Exp,
                                     accum_out=zcol[:])
                nc.vector.reciprocal(out=rz_sb[:, hp, ib:ib + 1], in_=zcol[:])
                nc.vector.tensor_scalar_mul(attn_sb[:, hp, ib, :],
                                            attn_sb[:, hp, ib, :],
                                            rz_sb[:, hp, ib:ib + 1])
            # attnT: [j, 6i+h] for i in this ib
            for hp in range(3):
                for jt in range(4):
                    pt = ps_t.tile([P, P], BF16, tag="pst")
                    nc.tensor.transpose(pt[:], attn_sb[:, hp, ib, jt * P:(jt + 1) * P],
                                        ident[:])
                    # pt rows j, cols (hs,i') -> attnT col 384*ib + 6*i' + 2hp + hs
                    at_base = attnT[:, jt, :]
                    dst = bass.AP(tensor=at_base.tensor,
                                  offset=at_base.offset + 384 * ib + 2 * hp,
                                  ap=[at_base.ap[0], [1, 2], [6, 64]])
                    pt_base = pt[:]
                    src = bass.AP(tensor=pt_base.tensor, offset=pt_base.offset,
                                  ap=[pt_base.ap[0], [64, 2], [1, 64]])
                    nc.vector.tensor_copy(out=dst, in_=src)
            # attn@rel for groups of this ib
            for gg in range(8):
                g = 8 * ib + gg
                rn = rn_tiles[gg]
                ar_ps = ps_rs.tile([P, N], F32, tag="rstream")
                for jt in range(4):
                    nc.tensor.matmul(ar_ps[0:48, :],
                                     attnT[:, jt, 48 * g:48 * g + 48],
                                     rn[:, jt, :, :], start=(jt == 0), stop=(jt == 3),
                                     skip_group_check=True)
                ar_sb = arel_pool.tile([48, N], BF16, tag="arsb")
                nc.vector.tensor_copy(out=ar_sb[:], in_=ar_ps[0:48, :])
                for ct in range(4):
                    pt = ps_t.tile([P, P], BF16, tag="pst")
                    nc.tensor.transpose(pt[:, 0:48], ar_sb[:, ct * P:(ct + 1) * P],
                                        ident[0:48, 0:48])
                    art = arel_pool.tile([P, 48], BF16, tag="artsb")
                    nc.scalar.copy(out=art[:], in_=pt[:, 0:48])
                    for nd in range(2):
                        n = 2 * ct + nd
                        i = 8 * g + n
                        blk = art[64 * nd:64 * nd + 64, 6 * n:6 * n + 6]
                        if n % 2 == 0:
                            nc.vector.tensor_copy(out=aoT_alt[:, :, i], in_=blk)
                        else:
                            nc.scalar.copy(out=aoT_alt[:, :, i], in_=blk)
            if ib % 2 == 1:
                emit_head_merge(ib // 2)

        # ---- LN2 + MLP ----
        layer_norm(x2_sb, ln2w_sb, ln2b_sb, x2n_sb, 2)
        for dt in range(3):
            for it in range(2):
                pt = ps_t.tile([P, P], BF16, tag="pst")
                nc.tensor.transpose(pt[:], x2n_sb[:, it, dt * P:(dt + 1) * P], ident[:])
                nc.vector.tensor_copy(out=x2nT[:, dt, it * P:(it + 1) * P], in_=pt[:])
        for ft in range(12):
            ps = ps_w.tile([P, N], F32, tag="pswork")
            for dt in range(3):
                nc.tensor.matmul(ps[:, 0:I], w1t_sb[:, dt, ft * P:(ft + 1) * P],
                                 x2nT[:, dt, :], start=(dt == 0), stop=(dt == 2))
            nc.scalar.activation(out=h1g[:, ft, :], in_=ps[:, 0:I],
                                 func=mybir.ActivationFunctionType.Gelu,
                                 bias=b1_sb[:, ft:ft + 1], scale=1.0)
        for it in range(2):
            ps = ps_w.tile([P, N], F32, tag="pswork")
            for ft in range(12):
                nc.tensor.matmul(ps[:, 0:D], h1g[:, ft, it * P:(it + 1) * P],
                                 w2t_sb[:, ft, :], start=(ft == 0), stop=(ft == 11))
            tmp = small.tile([P, D], F32, tag="res")
            nc.vector.tensor_tensor(out=tmp[:], in0=ps[:, 0:D], in1=b2_sb[:],
                                    op=mybir.AluOpType.add)
            nc.vector.tensor_tensor(out=out_sb[:, it, :], in0=tmp[:],
                                    in1=x2_sb[:, it, :], op=mybir.AluOpType.add)

        nc.sync.dma_start(out=out.ap().rearrange("(t p) d -> p t d", p=P),
                          in_=out_sb[:])

    nc.compile()
    return nc


_NC_CACHE = None


def _get_nc():
    global _NC_CACHE
    if _NC_CACHE is None:
        _NC_CACHE = build_nc()
    return _NC_CACHE


def kernel(x, rel_pos_bias, ln1_w, ln1_b, ln2_w, ln2_b, Wq, Wk, Wv, Wo, bo,
           W1, b1, W2, b2):
    nc = _get_nc()
    perm_f = _build_perm()
    common = {
        "wqt": np.ascontiguousarray(Wq.T).astype(_NP_BF16),
        "wkt": np.ascontiguousarray(Wk.T).astype(_NP_BF16),
        "wvt": np.ascontiguousarray(Wv.T).astype(_NP_BF16),
        "wot": np.ascontiguousarray(Wo.T).astype(_NP_BF16),
        "w1t": np.ascontiguousarray(W1.T).astype(_NP_BF16),
        "w2t": np.ascontiguousarray(W2.T).astype(_NP_BF16),
        "perm": perm_f.astype(_NP_BF16),
        "ln1w": np.asarray(ln1_w, np.float32), "ln1b": np.asarray(ln1_b, np.float32),
        "ln2w": np.asarray(ln2_w, np.float32), "ln2b": np.asarray(ln2_b, np.float32),
        "bo": np.asarray(bo, np.float32), "b1": np.asarray(b1, np.float32),
        "b2": np.asarray(b2, np.float32),
    }
    in_maps = []
    for core in range(NCORES):
        b, ih = core // 2, core % 2
        own = slice(ih * I, (ih + 1) * I)
        othr = slice((1 - ih) * I, (2 - ih) * I)
        permrows = np.r_[ih * I:(ih + 1) * I, (1 - ih) * I:(2 - ih) * I]
        xp = np.ascontiguousarray(np.asarray(x[b], np.float32)[permrows])
        rel = np.asarray(rel_pos_bias[b], np.float32)[own][:, permrows, :]
        rel_bf = rel.astype(_NP_BF16)
        relT = np.ascontiguousarray(rel_bf.transpose(0, 2, 1))
        relN = np.ascontiguousarray(
            rel_bf.transpose(1, 0, 2).reshape(4, P, I, HD))
        in_maps.append({**common, "xp": xp, "relT": relT, "relN": relN})
    res = run_bass_kernel_spmd(nc, in_maps, core_ids=list(range(NCORES)))
    out = np.empty((B, N, D), np.float32)
    for core in range(NCORES):
        b, ih = core // 2, core % 2
        out[b, ih * I:(ih + 1) * I] = res.results[core]["out"]
    return out
